# revision 4
# baseline (speedup 1.0000x reference)
"""Trainium2 Bass kernel for nn_Aggregator (GNN message-passing aggregation).

Computes, for N=16384 nodes with K=32 messages of dim D=256 each:
    out[n, :] = relu(curr_emb[n, 0, :] + sum_k alpha[n, k] * msg[n, k, :])

Strategy (memory-bound problem):
  - Data-parallel over nodes: 8 NeuronCores x 2048 nodes each.
  - Only slot 0 of curr_emb is read (host slices it; saves 496 MiB of traffic).
  - All device traffic is bf16 (rel err ~2e-3, 10x under the 2e-2 gate):
    host downcasts + packs msg/cur/alpha for one block of 128 nodes into a
    single contiguous [128, 8480] tile so each block is ONE full-bandwidth
    DMA (16.6 KiB contiguous per partition).
  - Per core, loop over 16 blocks of 128 nodes. The weighted sum runs on the
    TensorEngine as 32 block-diagonal matmuls per block accumulating into one
    PSUM tile (plus 4 identity-slice matmuls that seed PSUM with curr_emb):
      * moving operand of matmul g: msg of nodes 4g..4g+3 laid out as
        [128 partitions = (node%4, k), 256]
      * stationary operand: [128, 32] block-diagonal alpha built on the
        VectorEngine as one masks4 * alpha broadcast multiply per block
      * matmul g writes PSUM partitions 32*(g//8)..+32 via column tiling;
        node 128*b + p lands on PSUM partition p.
  - ScalarEngine applies relu reading PSUM (bf16 out), DMA stores, host
    upcasts the result to f32.
"""

import numpy as np

N, K, D = 16384, 32, 256
N_CORES = 8
NPC = N // N_CORES  # nodes per core
P = 128  # nodes per block (= partitions)
G = P // 4  # matmul groups per block (4 nodes each)
W = G * D + D + G  # packed tile width: msg | cur | alpha
CUR_OFF = G * D
AL_OFF = G * D + D

_cache: dict = {}


def _split_excess_waits(nc, max_waits: int = 1) -> int:
    """This container's walrus rejects >1 sync-wait per instruction
    ("Too many sync wait commands"). TileContext attaches several to the
    kernel-tail drain. Hoist the excess onto NoOps injected just before the
    instruction on the same engine (sequential waits == multi-wait)."""
    import bass_rust
    from concourse import mybir

    n_split = 0
    for fn in nc.m.functions:
        for bb in fn.blocks:
            out = []
            for inst in bb.instructions:
                si = inst.sync_info
                waits = list(si.on_wait) if si is not None else []
                if len(waits) > max_waits:
                    keep = waits[-max_waits:]
                    excess = waits[:-max_waits]
                    for i0 in range(0, len(excess), max_waits):
                        nop = mybir.InstNoOp(
                            name=f"{inst.name}-wsplit{i0}", ins=[], outs=[]
                        )
                        nop.engine = inst.engine
                        nop.sync_info = bass_rust.SyncInfo(
                            on_wait=excess[i0 : i0 + max_waits], on_update=[]
                        )
                        out.append(nop)
                        n_split += 1
                    inst.sync_info = bass_rust.SyncInfo(
                        on_wait=keep, on_update=list(si.on_update)
                    )
                out.append(inst)
            bb.instructions = out
    return n_split


def build_nc(npc: int = NPC, data_bufs: int = 3, fix_waits: bool = True, repeats: int = 1):
    """Build the single-core Bass program (replicated SPMD across 8 cores)."""
    import concourse.bass as bass
    import concourse.tile as tile
    from concourse import mybir

    f32 = mybir.dt.float32
    bf16 = mybir.dt.bfloat16
    nb = npc // P  # node blocks

    nc = bass.Bass("TRN2", target_bir_lowering=False, debug=False, num_devices=N_CORES)

    qdata_d = nc.dram_tensor("qdata", [nb, P, W], bf16, kind="ExternalInput").ap()
    ident_d = nc.dram_tensor("ident", [P, P], bf16, kind="ExternalInput").ap()
    masks4_d = nc.dram_tensor("masks4", [P, G, 32], bf16, kind="ExternalInput").ap()
    out_d = nc.dram_tensor("out", [npc, D], bf16, kind="ExternalOutput").ap()

    with tile.TileContext(nc) as tc:
        with (
            tc.tile_pool(name="const", bufs=1) as const_pool,
            tc.tile_pool(name="data", bufs=data_bufs) as data_pool,
            tc.tile_pool(name="w", bufs=2) as w_pool,
            tc.tile_pool(name="o", bufs=3) as o_pool,
            tc.tile_pool(name="ps", bufs=2, space="PSUM") as ps_pool,
        ):
            ident_t = const_pool.tile([P, P], bf16)
            nc.scalar.dma_start(ident_t[:], ident_d[:])
            mask_t = const_pool.tile([P, G, 32], bf16)
            nc.scalar.dma_start(mask_t[:], masks4_d[:])

            for b in [bb for _ in range(repeats) for bb in range(nb)]:
                dat_t = data_pool.tile([P, W], bf16)
                nc.sync.dma_start(dat_t[:], qdata_d[b])

                # w[p, g, c] = masks4[p, g, c] * alpha[p, g]  (one DVE op;
                # alpha column is broadcast over c via a 0-stride AP)
                w_t = w_pool.tile([P, G, 32], bf16)
                a_ap = dat_t[:, AL_OFF : AL_OFF + G]
                a_bcast = bass.AP(
                    a_ap.tensor, a_ap.offset, [list(a_ap.ap[0]), [1, G], [0, 32]]
                )
                nc.vector.tensor_tensor(
                    w_t[:], mask_t[:], a_bcast, mybir.AluOpType.mult
                )

                ps_t = ps_pool.tile([P, D], f32)
                for cg in range(4):
                    # seed PSUM partitions 32cg..32cg+32 with curr_emb rows
                    nc.tensor.matmul(
                        ps_t[32 * cg : 32 * (cg + 1), :],
                        ident_t[:, 32 * cg : 32 * (cg + 1)],
                        dat_t[:, CUR_OFF : CUR_OFF + D],
                        start=True,
                        stop=False,
                        tile_position=(0, 32 * cg),
                    )
                    for g in range(8 * cg, 8 * cg + 8):
                        nc.tensor.matmul(
                            ps_t[32 * cg : 32 * (cg + 1), :],
                            w_t[:, g, :],
                            dat_t[:, D * g : D * (g + 1)],
                            start=False,
                            stop=(g % 8 == 7),
                            tile_position=(0, 32 * cg),
                        )

                o_t = o_pool.tile([P, D], bf16)
                nc.scalar.activation(
                    o_t[:], ps_t[:], mybir.ActivationFunctionType.Relu
                )
                nc.scalar.dma_start(out_d[b * P : (b + 1) * P, :], o_t[:])

    if fix_waits:
        _split_excess_waits(nc)
    return nc


def _host_prep(curr_emb, alpha, msg, npc):
    """Downcast to bf16, shard + repack host-side. Returns per-core inputs."""
    import ml_dtypes

    bf = ml_dtypes.bfloat16
    nb = npc // P
    n = npc * N_CORES

    # msg -> [core, b, p=(j*K+k), g*D+d]
    mq = np.asarray(msg, dtype=np.float32).astype(bf)
    mq = mq.reshape(N_CORES, nb, G, 4, K, D).transpose(0, 1, 3, 4, 2, 5)
    mq = np.ascontiguousarray(mq).reshape(N_CORES, nb, P, G * D)

    cq = np.asarray(curr_emb[:, 0, :], dtype=np.float32).astype(bf)
    cq = cq.reshape(N_CORES, nb, P, D)

    # alpha -> [core, b, p=(j*K+k), g]
    aq = np.asarray(alpha, dtype=np.float32).reshape(n, K).astype(bf)
    aq = aq.reshape(N_CORES, nb, G, 4, K).transpose(0, 1, 3, 4, 2)
    aq = np.ascontiguousarray(aq).reshape(N_CORES, nb, P, G)

    qdata = np.concatenate([mq, cq, aq], axis=3)

    ident = np.eye(P, dtype=np.float32).astype(bf)
    masks4 = np.zeros((P, G, 32), dtype=np.float32)
    for g in range(G):
        c = g % 8
        for j in range(4):
            masks4[32 * j : 32 * (j + 1), g, 4 * c + j] = 1.0
    masks4 = masks4.astype(bf)

    return [
        {"qdata": qdata[core], "ident": ident, "masks4": masks4}
        for core in range(N_CORES)
    ]


def kernel(curr_emb, alpha, msg):
    from concourse.bass_utils import run_bass_kernel_spmd

    if "nc" not in _cache:
        _cache["nc"] = build_nc()
    nc = _cache["nc"]
    in_maps = _host_prep(curr_emb, alpha, msg, NPC)
    res = run_bass_kernel_spmd(nc, in_maps, list(range(N_CORES)))
    out = np.concatenate([res.results[i]["out"] for i in range(N_CORES)], axis=0)
    return out.astype(np.float32)


# revision 5
# speedup vs baseline: 1.8728x; 1.8728x over previous
"""Trainium2 Bass kernel for nn_Aggregator (GNN message-passing aggregation).

Computes, for N=16384 nodes with K=32 messages of dim D=256 each:
    out[n, :] = relu(curr_emb[n, 0, :] + sum_k alpha[n, k] * msg[n, k, :])

Strategy (memory-bound problem):
  - Data-parallel over nodes: 8 NeuronCores x 2048 nodes each.
  - Only slot 0 of curr_emb is read (host slices it; saves 496 MiB of traffic).
  - Mixed-precision message traffic, exploiting that the sum is tolerant to
    quantization of low-weight terms: per node the host sorts messages by
    alpha; the 8 largest-alpha messages ship as bf16, the other 24 as
    fp8 e3m4 (4 mantissa bits). alpha/cur ship as bf16. Measured rel err
    ~1.2e-2 vs the 2e-2 gate. 1.25 bytes/elem vs 4 for f32.
  - Per core, loop over 16 blocks of 128 nodes; per block TWO contiguous
    full-bandwidth DMAs (one bf16 tile: top-8 msg + cur + alphas; one fp8
    tile: bottom-24 msg).
  - The weighted sum runs on the TensorEngine as 32 block-diagonal matmuls
    per block accumulating into one PSUM tile (plus 4 identity-slice matmuls
    seeding PSUM with curr_emb). Sections pack the contraction dim as
    (node-in-group j, k-slot) = 128 partitions:
      S0 bf16  k-slots 0..8   -> 8 groups of 16 nodes (j=16, kh=8)
      S1 e3m4  k-slots 8..24  -> 16 groups of 8 nodes (j=8, kh=16)
      S2 e3m4  k-slots 24..32 -> 8 groups of 16 nodes (j=16, kh=8)
    Stationary operands are [128, 32] block-diagonal alpha tiles (bf16),
    built per block by ONE VectorEngine multiply: masks * alpha-broadcast.
    Matmul for group g writes PSUM partitions 32*cg..+32 via column tiling;
    node 128*b + p lands on PSUM partition p.
  - ScalarEngine applies relu reading PSUM (bf16 out), DMA stores, host
    upcasts the result to f32.
"""

import numpy as np

N, K, D = 16384, 32, 256
N_CORES = 8
NPC = N // N_CORES  # nodes per core
P = 128  # nodes per block (= partitions)

TOP = 8  # k-slots shipped as bf16
# (section k-slot count, nodes per group) for the bf16 then fp8 tiles
S0 = (8, 16)  # bf16: slots 0..8
S1 = (16, 8)  # e3m4: slots 8..24
S2 = (8, 16)  # e3m4: slots 24..32
NG0, NG1, NG2 = P // S0[1], P // S1[1], P // S2[1]  # 8, 16, 8 groups
BW = NG0 * D + D + 32  # bf16 tile: msg | cur | a_all
CUR_OFF = NG0 * D
AL_OFF = NG0 * D + D
FW = (NG1 + NG2) * D  # fp8 tile: S1 msg | S2 msg
S2_OFF = NG1 * D

_cache: dict = {}


def _split_excess_waits(nc, max_waits: int = 1) -> int:
    """This container's walrus rejects >1 sync-wait per instruction
    ("Too many sync wait commands"). TileContext attaches several to the
    kernel-tail drain. Hoist the excess onto NoOps injected just before the
    instruction on the same engine (sequential waits == multi-wait)."""
    import bass_rust
    from concourse import mybir

    n_split = 0
    for fn in nc.m.functions:
        for bb in fn.blocks:
            out = []
            for inst in bb.instructions:
                si = inst.sync_info
                waits = list(si.on_wait) if si is not None else []
                if len(waits) > max_waits:
                    keep = waits[-max_waits:]
                    excess = waits[:-max_waits]
                    for i0 in range(0, len(excess), max_waits):
                        nop = mybir.InstNoOp(
                            name=f"{inst.name}-wsplit{i0}", ins=[], outs=[]
                        )
                        nop.engine = inst.engine
                        nop.sync_info = bass_rust.SyncInfo(
                            on_wait=excess[i0 : i0 + max_waits], on_update=[]
                        )
                        out.append(nop)
                        n_split += 1
                    inst.sync_info = bass_rust.SyncInfo(
                        on_wait=keep, on_update=list(si.on_update)
                    )
                out.append(inst)
            bb.instructions = out
    return n_split


def _band_groups(cg):
    """(w-group index gg, section id) for PSUM band cg, emission order."""
    out = []
    for r0 in (2 * cg, 2 * cg + 1):
        out.append((r0, 0))
    for r1 in range(4 * cg, 4 * cg + 4):
        out.append((NG0 + r1, 1))
    for r2 in (2 * cg, 2 * cg + 1):
        out.append((NG0 + NG1 + r2, 2))
    return out


def build_nc(npc: int = NPC, bufs: int = 3, fix_waits: bool = True, repeats: int = 1):
    """Build the single-core Bass program (replicated SPMD across 8 cores)."""
    import concourse.bass as bass
    import concourse.tile as tile
    from concourse import mybir

    f32 = mybir.dt.float32
    bf16 = mybir.dt.bfloat16
    f8e3 = mybir.dt.float8e3
    nb = npc // P  # node blocks

    nc = bass.Bass("TRN2", target_bir_lowering=False, debug=False, num_devices=N_CORES)

    qb_d = nc.dram_tensor("qb", [nb, P, BW], bf16, kind="ExternalInput").ap()
    qf_d = nc.dram_tensor("qf", [nb, P, FW], f8e3, kind="ExternalInput").ap()
    ident_d = nc.dram_tensor("ident", [P, P], bf16, kind="ExternalInput").ap()
    masks_d = nc.dram_tensor("masks", [P, 32, 32], bf16, kind="ExternalInput").ap()
    out_d = nc.dram_tensor("out", [npc, D], bf16, kind="ExternalOutput").ap()

    with tile.TileContext(nc) as tc:
        with (
            tc.tile_pool(name="const", bufs=1) as const_pool,
            tc.tile_pool(name="qb", bufs=bufs) as qb_pool,
            tc.tile_pool(name="qf", bufs=bufs) as qf_pool,
            tc.tile_pool(name="w", bufs=2) as w_pool,
            tc.tile_pool(name="o", bufs=3) as o_pool,
            tc.tile_pool(name="ps", bufs=2, space="PSUM") as ps_pool,
        ):
            ident_t = const_pool.tile([P, P], bf16)
            nc.scalar.dma_start(ident_t[:], ident_d[:])
            mask_t = const_pool.tile([P, 32, 32], bf16)
            nc.scalar.dma_start(mask_t[:], masks_d[:])

            for b in [bb for _ in range(repeats) for bb in range(nb)]:
                qb_t = qb_pool.tile([P, BW], bf16)
                nc.sync.dma_start(qb_t[:], qb_d[b])
                qf_t = qf_pool.tile([P, FW], f8e3)
                nc.sync.dma_start(qf_t[:], qf_d[b])

                # w[p, gg, c] = masks[p, gg, c] * a_all[p, gg]  (one DVE op;
                # the alpha column is broadcast over c via a 0-stride AP)
                w_t = w_pool.tile([P, 32, 32], bf16)
                a_ap = qb_t[:, AL_OFF : AL_OFF + 32]
                a_bcast = bass.AP(
                    a_ap.tensor, a_ap.offset, [list(a_ap.ap[0]), [1, 32], [0, 32]]
                )
                nc.vector.tensor_tensor(
                    w_t[:], mask_t[:], a_bcast, mybir.AluOpType.mult
                )

                ps_t = ps_pool.tile([P, D], f32)
                for cg in range(4):
                    # seed PSUM partitions 32cg..32cg+32 with curr_emb rows
                    nc.tensor.matmul(
                        ps_t[32 * cg : 32 * (cg + 1), :],
                        ident_t[:, 32 * cg : 32 * (cg + 1)],
                        qb_t[:, CUR_OFF : CUR_OFF + D],
                        start=True,
                        stop=False,
                        tile_position=(0, 32 * cg),
                    )
                    groups = _band_groups(cg)
                    for i, (gg, sec) in enumerate(groups):
                        if sec == 0:
                            mv = qb_t[:, D * gg : D * (gg + 1)]
                        elif sec == 1:
                            r1 = gg - NG0
                            mv = qf_t[:, D * r1 : D * (r1 + 1)]
                        else:
                            r2 = gg - NG0 - NG1
                            mv = qf_t[:, S2_OFF + D * r2 : S2_OFF + D * (r2 + 1)]
                        nc.tensor.matmul(
                            ps_t[32 * cg : 32 * (cg + 1), :],
                            w_t[:, gg, :],
                            mv,
                            start=False,
                            stop=(i == len(groups) - 1),
                            tile_position=(0, 32 * cg),
                        )

                o_t = o_pool.tile([P, D], bf16)
                nc.scalar.activation(
                    o_t[:], ps_t[:], mybir.ActivationFunctionType.Relu
                )
                nc.scalar.dma_start(out_d[b * P : (b + 1) * P, :], o_t[:])

    if fix_waits:
        _split_excess_waits(nc)
    return nc


def _sec_pack(arr, ng, jn, kh, nb):
    """[cores*npc, kh, D] -> [cores, nb, P=(j*kh+k), ng*D] for a section."""
    c = N_CORES
    a = arr.reshape(c, nb, ng, jn, kh, D).transpose(0, 1, 3, 4, 2, 5)
    return np.ascontiguousarray(a).reshape(c, nb, P, ng * D)


def _a_pack(al, ng, jn, kh, nb):
    """[cores*npc, kh] -> [cores, nb, P, ng] alpha columns for a section."""
    c = N_CORES
    a = al.reshape(c, nb, ng, jn, kh).transpose(0, 1, 3, 4, 2)
    return np.ascontiguousarray(a).reshape(c, nb, P, ng)


def _host_prep(curr_emb, alpha, msg, npc):
    """Sort by alpha, downcast, shard + repack. Returns per-core inputs."""
    import ml_dtypes

    bf = ml_dtypes.bfloat16
    f8 = ml_dtypes.float8_e3m4
    nb = npc // P
    n = npc * N_CORES

    al = np.asarray(alpha, dtype=np.float32).reshape(n, K)
    order = np.argsort(-al, axis=1, kind="stable")  # [N, K] descending alpha
    al_s = np.take_along_axis(al, order, axis=1).astype(bf)
    msg = np.asarray(msg, dtype=np.float32)

    kh0, kh1, kh2 = S0[0], S1[0], S2[0]
    top = np.take_along_axis(msg, order[:, :kh0, None], axis=1).astype(bf)
    mid = np.take_along_axis(msg, order[:, kh0 : kh0 + kh1, None], axis=1).astype(f8)
    bot = np.take_along_axis(msg, order[:, kh0 + kh1 :, None], axis=1).astype(f8)

    qb0 = _sec_pack(top, NG0, S0[1], kh0, nb)  # [C, nb, P, NG0*D] bf16
    cq = np.asarray(curr_emb[:, 0, :], dtype=np.float32).astype(bf)
    cq = cq.reshape(N_CORES, nb, P, D)
    a0 = _a_pack(al_s[:, :kh0], NG0, S0[1], kh0, nb)
    a1 = _a_pack(al_s[:, kh0 : kh0 + kh1], NG1, S1[1], kh1, nb)
    a2 = _a_pack(al_s[:, kh0 + kh1 :], NG2, S2[1], kh2, nb)
    qb = np.concatenate([qb0, cq, a0, a1, a2], axis=3)

    qf = np.concatenate(
        [_sec_pack(mid, NG1, S1[1], kh1, nb), _sec_pack(bot, NG2, S2[1], kh2, nb)],
        axis=3,
    )

    ident = np.eye(P, dtype=np.float32).astype(bf)
    masks = np.zeros((P, 32, 32), dtype=np.float32)
    p = np.arange(P)
    for gg in range(32):
        if gg < NG0:
            r, col = gg, 16 * (gg % 2) + p // 8
        elif gg < NG0 + NG1:
            r = gg - NG0
            col = 8 * (r % 4) + p // 16
        else:
            r = gg - NG0 - NG1
            col = 16 * (r % 2) + p // 8
        masks[p, gg, col] = 1.0
    masks = masks.astype(bf)

    return [
        {"qb": qb[core], "qf": qf[core], "ident": ident, "masks": masks}
        for core in range(N_CORES)
    ]


def kernel(curr_emb, alpha, msg):
    from concourse.bass_utils import run_bass_kernel_spmd

    if "nc" not in _cache:
        _cache["nc"] = build_nc()
    nc = _cache["nc"]
    in_maps = _host_prep(curr_emb, alpha, msg, NPC)
    res = run_bass_kernel_spmd(nc, in_maps, list(range(N_CORES)))
    out = np.concatenate([res.results[i]["out"] for i in range(N_CORES)], axis=0)
    return out.astype(np.float32)


# revision 6
# speedup vs baseline: 2.1397x; 1.1425x over previous
"""Trainium2 Bass kernel for nn_Aggregator (GNN message-passing aggregation).

Computes, for N=16384 nodes with K=32 messages of dim D=256 each:
    out[n, :] = relu(curr_emb[n, 0, :] + sum_k alpha[n, k] * msg[n, k, :])

Strategy (memory-bound problem):
  - Data-parallel over nodes: 8 NeuronCores x 2048 nodes each.
  - Only slot 0 of curr_emb is read (host slices it; saves 496 MiB of traffic).
  - Mixed-precision message traffic, exploiting that the sum is tolerant to
    quantization of low-weight terms: per node the host sorts messages by
    alpha; the largest-alpha messages ship as bf16, the rest as fp8 e3m4
    (4 mantissa bits). alpha/cur ship as bf16. Measured rel err ~1.2e-2 vs
    the 2e-2 gate.
  - Per core, loop over 16 blocks of 128 nodes; per block TWO contiguous
    full-bandwidth DMAs (bf16 tile: top msg + cur + alphas; fp8 tile: rest).
  - The weighted sum runs on the TensorEngine as 32 block-diagonal matmuls
    per block accumulating into one PSUM tile. A section with kh k-slots
    packs the contraction dim as (node-in-group j=128/kh, k-slot) = 128
    partitions and contributes kh matmul groups of j nodes each; sections'
    kh sum to 32 slots = 32 matmuls. Stationary operands are [128, 32]
    block-diagonal alpha tiles (bf16), built per block by ONE VectorEngine
    multiply (masks * alpha-broadcast). The matmul for a group covering
    nodes j*r..j*r+j writes PSUM partitions 32cg..32cg+32 (cg = j*r//32)
    via column tiling; node 128*b + p lands on PSUM partition p.
  - VectorEngine adds curr_emb to the PSUM result (cheaper than identity
    matmul seeds: PE streaming is co-bottleneck), ScalarEngine applies relu
    (bf16 out), DMA stores, host upcasts the result to f32.
"""

import numpy as np

N, K, D = 16384, 32, 256
N_CORES = 8
NPC = N // N_CORES  # nodes per core
P = 128  # nodes per block (= partitions)

# (dtype tag, k-slot count) per section, in sorted-alpha order; kh sums to 32.
SECTIONS = [("bf", 4), ("f8", 16), ("f8", 8), ("f8", 4)]

BF_MSG = sum(kh for dt, kh in SECTIONS if dt == "bf")
F8_MSG = sum(kh for dt, kh in SECTIONS if dt == "f8")
BW = BF_MSG * D + D + 32  # bf16 tile: bf msg | cur | a_all(32)
CUR_OFF = BF_MSG * D
AL_OFF = BF_MSG * D + D
FW = F8_MSG * D  # fp8 tile: f8 msg sections

_cache: dict = {}


def _split_excess_waits(nc, max_waits: int = 1) -> int:
    """This container's walrus rejects >1 sync-wait per instruction
    ("Too many sync wait commands"). TileContext attaches several to the
    kernel-tail drain. Hoist the excess onto NoOps injected just before the
    instruction on the same engine (sequential waits == multi-wait)."""
    import bass_rust
    from concourse import mybir

    n_split = 0
    for fn in nc.m.functions:
        for bb in fn.blocks:
            out = []
            for inst in bb.instructions:
                si = inst.sync_info
                waits = list(si.on_wait) if si is not None else []
                if len(waits) > max_waits:
                    keep = waits[-max_waits:]
                    excess = waits[:-max_waits]
                    for i0 in range(0, len(excess), max_waits):
                        nop = mybir.InstNoOp(
                            name=f"{inst.name}-wsplit{i0}", ins=[], outs=[]
                        )
                        nop.engine = inst.engine
                        nop.sync_info = bass_rust.SyncInfo(
                            on_wait=excess[i0 : i0 + max_waits], on_update=[]
                        )
                        out.append(nop)
                        n_split += 1
                    inst.sync_info = bass_rust.SyncInfo(
                        on_wait=keep, on_update=list(si.on_update)
                    )
                out.append(inst)
            bb.instructions = out
    return n_split


def _sec_layout(sections):
    """Per section: (dtype, kh, j, gg offset, tile col offset in its tile)."""
    out, gg0, bfc, f8c = [], 0, 0, 0
    for dt, kh in sections:
        col = bfc if dt == "bf" else f8c
        out.append((dt, kh, P // kh, gg0, col))
        gg0 += kh
        if dt == "bf":
            bfc += kh * D
        else:
            f8c += kh * D
    return out


def _band_groups(cg, sections):
    """(gg, dtype, moving col offset) for PSUM band cg, emission order."""
    out = []
    for dt, kh, j, gg0, col0 in _sec_layout(sections):
        for r in range(32 * cg // j, 32 * (cg + 1) // j):
            out.append((gg0 + r, dt, col0 + r * D))
    return out


def build_nc(
    npc: int = NPC,
    bufs: int = 3,
    fix_waits: bool = True,
    repeats: int = 1,
    sections=None,
):
    """Build the single-core Bass program (replicated SPMD across 8 cores)."""
    import concourse.bass as bass
    import concourse.tile as tile
    from concourse import mybir

    if sections is None:
        sections = SECTIONS
    f32 = mybir.dt.float32
    bf16 = mybir.dt.bfloat16
    f8e3 = mybir.dt.float8e3
    nb = npc // P  # node blocks

    nc = bass.Bass("TRN2", target_bir_lowering=False, debug=False, num_devices=N_CORES)

    qb_d = nc.dram_tensor("qb", [nb, P, BW], bf16, kind="ExternalInput").ap()
    qf_d = nc.dram_tensor("qf", [nb, P, FW], f8e3, kind="ExternalInput").ap()
    masks_d = nc.dram_tensor("masks", [P, 32, 32], bf16, kind="ExternalInput").ap()
    out_d = nc.dram_tensor("out", [npc, D], bf16, kind="ExternalOutput").ap()

    with tile.TileContext(nc) as tc:
        with (
            tc.tile_pool(name="const", bufs=1) as const_pool,
            tc.tile_pool(name="qb", bufs=bufs) as qb_pool,
            tc.tile_pool(name="qf", bufs=bufs) as qf_pool,
            tc.tile_pool(name="w", bufs=2) as w_pool,
            tc.tile_pool(name="t", bufs=3) as t_pool,
            tc.tile_pool(name="o", bufs=3) as o_pool,
            tc.tile_pool(name="ps", bufs=2, space="PSUM") as ps_pool,
        ):
            mask_t = const_pool.tile([P, 32, 32], bf16)
            nc.scalar.dma_start(mask_t[:], masks_d[:])

            for b in [bb for _ in range(repeats) for bb in range(nb)]:
                qb_t = qb_pool.tile([P, BW], bf16)
                nc.sync.dma_start(qb_t[:], qb_d[b])
                qf_t = qf_pool.tile([P, FW], f8e3)
                nc.sync.dma_start(qf_t[:], qf_d[b])

                # w[p, gg, c] = masks[p, gg, c] * a_all[p, gg]  (one DVE op;
                # the alpha column is broadcast over c via a 0-stride AP)
                w_t = w_pool.tile([P, 32, 32], bf16)
                a_ap = qb_t[:, AL_OFF : AL_OFF + 32]
                a_bcast = bass.AP(
                    a_ap.tensor, a_ap.offset, [list(a_ap.ap[0]), [1, 32], [0, 32]]
                )
                nc.vector.tensor_tensor(
                    w_t[:], mask_t[:], a_bcast, mybir.AluOpType.mult
                )

                ps_t = ps_pool.tile([P, D], f32)
                for cg in range(4):
                    groups = _band_groups(cg, sections)
                    for i, (gg, dt, col) in enumerate(groups):
                        mv = (qb_t if dt == "bf" else qf_t)[:, col : col + D]
                        nc.tensor.matmul(
                            ps_t[32 * cg : 32 * (cg + 1), :],
                            w_t[:, gg, :],
                            mv,
                            start=(i == 0),
                            stop=(i == len(groups) - 1),
                            tile_position=(0, 32 * cg),
                        )

                # t = ps + cur on DVE, then relu on ScalarEngine
                t_t = t_pool.tile([P, D], f32)
                nc.vector.tensor_tensor(
                    t_t[:], ps_t[:], qb_t[:, CUR_OFF : CUR_OFF + D],
                    mybir.AluOpType.add,
                )
                o_t = o_pool.tile([P, D], bf16)
                nc.scalar.activation(
                    o_t[:], t_t[:], mybir.ActivationFunctionType.Relu
                )
                nc.scalar.dma_start(out_d[b * P : (b + 1) * P, :], o_t[:])

    if fix_waits:
        _split_excess_waits(nc)
    return nc


def _sec_pack(arr, kh, nb):
    """[cores*npc, kh, D] -> [cores, nb, P=(j-idx*kh + k), kh*D]."""
    c, j = N_CORES, P // kh
    a = arr.reshape(c, nb, kh, j, kh, D).transpose(0, 1, 3, 4, 2, 5)
    return np.ascontiguousarray(a).reshape(c, nb, P, kh * D)


def _a_pack(al, kh, nb):
    """[cores*npc, kh] -> [cores, nb, P, kh] alpha columns for a section."""
    c, j = N_CORES, P // kh
    a = al.reshape(c, nb, kh, j, kh).transpose(0, 1, 3, 4, 2)
    return np.ascontiguousarray(a).reshape(c, nb, P, kh)


def _host_prep(curr_emb, alpha, msg, npc):
    """Sort by alpha, downcast, shard + repack. Returns per-core inputs."""
    import ml_dtypes

    bf = ml_dtypes.bfloat16
    f8 = ml_dtypes.float8_e3m4
    nb = npc // P
    n = npc * N_CORES

    al = np.asarray(alpha, dtype=np.float32).reshape(n, K)
    order = np.argsort(-al, axis=1, kind="stable")  # [N, K] descending alpha
    al_s = np.take_along_axis(al, order, axis=1).astype(bf)
    msg = np.asarray(msg, dtype=np.float32)

    bf_parts, f8_parts, a_parts = [], [], []
    k0 = 0
    for dt, kh in SECTIONS:
        sl = np.take_along_axis(msg, order[:, k0 : k0 + kh, None], axis=1)
        if dt == "bf":
            bf_parts.append(_sec_pack(sl.astype(bf), kh, nb))
        else:
            f8_parts.append(_sec_pack(sl.astype(f8), kh, nb))
        a_parts.append(_a_pack(al_s[:, k0 : k0 + kh], kh, nb))
        k0 += kh

    cq = np.asarray(curr_emb[:, 0, :], dtype=np.float32).astype(bf)
    cq = cq.reshape(N_CORES, nb, P, D)
    qb = np.concatenate(bf_parts + [cq] + a_parts, axis=3)
    qf = np.concatenate(f8_parts, axis=3)

    masks = np.zeros((P, 32, 32), dtype=np.float32)
    p = np.arange(P)
    for dt, kh, j, gg0, _ in _sec_layout(SECTIONS):
        for r in range(kh):
            masks[p, gg0 + r, (j * r) % 32 + p // kh] = 1.0
    masks = masks.astype(bf)

    return [
        {"qb": qb[core], "qf": qf[core], "masks": masks}
        for core in range(N_CORES)
    ]


def kernel(curr_emb, alpha, msg):
    from concourse.bass_utils import run_bass_kernel_spmd

    if "nc" not in _cache:
        _cache["nc"] = build_nc()
    nc = _cache["nc"]
    in_maps = _host_prep(curr_emb, alpha, msg, NPC)
    res = run_bass_kernel_spmd(nc, in_maps, list(range(N_CORES)))
    out = np.concatenate([res.results[i]["out"] for i in range(N_CORES)], axis=0)
    return out.astype(np.float32)
